# revision 3
# baseline (speedup 1.0000x reference)
# Trainium2 Bass kernel for AtomTypeGNN message passing.
#
#   adj_exp[m,k] = sum_n dist_adj[m,n] * dist_exp[m,n,k]          (streams 1 GiB)
#   feat[m,o]    = sum_{f,h} adj_exp[m,f] * w[f,h,o] * emb[m,h]
#   out          = softplus(feat) + b
#
# Output row m depends only on row m of the inputs -> pure data parallel over
# atoms, 8 NeuronCores, 256 atoms each, no collectives.
#
# Device strategy (per core), v2:
#   Step 1: atoms stream in groups of 8; one 2 MiB DMA per group on the sync
#     (SP HWDGE) queue, which carries NOTHING else, so no cross-phase
#     dependency ever stalls the exp stream (v1 lost ~15us to the aexp load
#     parked between blocks in the SP FIFO).  All 8 atoms of a group
#     accumulate into ONE PSUM bank at different free offsets: start=True on
#     the group's first matmul marks the whole 2 KiB zero region pending-zero,
#     so each atom's first chunk writes and later chunks accumulate
#     (has_written semantics).  One ScalarE evac [1,512] per group replaces
#     v1's four [1,64] copies (ScalarE busy ~115us -> ~22us).
#   Step 2: per 128-atom block, ONE scratch store + ONE gather load (both on
#     the gpsimd SWDGE queue) transpose adj_exp to [128 atoms, 64].  The
#     f-contraction feat = sum_f aexp[:,f] * G_f runs as FOUR interleaved
#     DVE scalar_tensor_tensor chains (dep distance 4 hides DVE latency);
#     G_f = emb @ w[f] is computed on the PE during the stream.  Softplus
#     splits between ScalarE (abs/exp/ln in one act table, pre-warmed at
#     kernel start so no table load lands in the tail) and DVE (min/relu/
#     adds).  Output DMAs ride the scalar (ACT HWDGE) queue: gpsimd's
#     end-of-kernel DRAIN cost ~7us in v1.
#
# Inputs are pre-swizzled/cast on the host (bf16 compute, f32 accumulate:
# ~3e-3 relative error, memory roofline halves to ~190us/core).

import numpy as np
import ml_dtypes

N = 2048
K = 64
H = 128
OUT = 128
N_CORES = 8
M = N // N_CORES  # 256 atoms per core
GA = 8            # atoms per group / per PSUM bank
NG = M // GA      # 32 groups per core
NBLK = M // 128   # 2 step-2 blocks per core

_BF = ml_dtypes.bfloat16

_CACHE = {}


def _ensure_path():
    import sys

    for p in ("/opt/trn_rl_repo",):
        if p not in sys.path:
            sys.path.insert(0, p)


def _build():
    _ensure_path()
    import concourse.bass as bass  # noqa: F401
    import concourse.tile as tile
    from concourse import bacc, mybir

    f32 = mybir.dt.float32
    bf16 = mybir.dt.bfloat16

    nc = bacc.Bacc(
        "TRN2",
        target_bir_lowering=False,
        debug=False,
        num_devices=N_CORES,
    )

    # [t, p, aq]: atom group t = atoms 8t..8t+7, partition p, aq = 1024*a + q,
    # q = 64*c + k, n = 16p + c.  Per partition 16 KiB contiguous in DRAM.
    exp_d = nc.declare_dram_parameter("exp", [NG, 128, 8 * 1024], bf16, isOutput=False)
    # adjA[j, 16m + c] = dist_adj[m, 16j + c]
    adjA_d = nc.declare_dram_parameter("adjA", [128, 16 * M], bf16, isOutput=False)
    # embT[h, m]
    embT_d = nc.declare_dram_parameter("embT", [H, M], bf16, isOutput=False)
    # w2[h, 128f + o] = bilinear_w[f, h, o]
    w_d = nc.declare_dram_parameter("w", [H, K * OUT], bf16, isOutput=False)
    # bias broadcast to all partitions
    bias_d = nc.declare_dram_parameter("bias", [128, OUT], f32, isOutput=False)
    out_d = nc.declare_dram_parameter("out", [M, OUT], f32, isOutput=True)

    # adj_exp bounce buffer, one row per block: [g*512 + a*64 + k]
    scratch_d = nc.dram_tensor("scratch", [NBLK, 16 * 512], f32)

    with tile.TileContext(nc) as tc:
        with (
            tc.tile_pool(name="const", bufs=1) as constp,
            tc.tile_pool(name="exp", bufs=5) as expp,
            tc.tile_pool(name="ps1", bufs=3, space="PSUM") as ps1p,
            tc.tile_pool(name="stage", bufs=1) as stagep,
            tc.tile_pool(name="aexp", bufs=2) as aexpp,
            tc.tile_pool(name="ps2", bufs=3, space="PSUM") as ps2p,
            tc.tile_pool(name="gsb", bufs=2) as gsbp,
            tc.tile_pool(name="acc", bufs=10) as accp,
            tc.tile_pool(name="outp", bufs=6) as outp,
        ):
            # consts on the scalar (ACT HWDGE) queue: parallel to the exp
            # stream, fast ramp.  bias first so the act-table warm-up can
            # issue immediately.
            biassb = constp.tile([128, OUT], f32, tag="bias")
            nc.scalar.dma_start(biassb[:], bias_d[:, :])
            adjA = constp.tile([128, 16 * M], bf16, tag="adjA")
            nc.scalar.dma_start(adjA[:], adjA_d[:, :])
            wsb = constp.tile([128, K * OUT], bf16, tag="wsb")
            nc.scalar.dma_start(wsb[:], w_d[:, :])
            embT = constp.tile([128, M], bf16, tag="embT")
            nc.scalar.dma_start(embT[:], embT_d[:, :])

            # Warm the natural_log_exp act table (abs/exp/ln/relu/copy share
            # it) before the first evac copy, so no ACT_TABLE_LOAD lands in
            # the tail's critical path.
            warm = constp.tile([1, 2], f32, tag="warm")
            nc.scalar.activation(
                warm[0:1, :], biassb[0:1, 0:2], mybir.ActivationFunctionType.Abs
            )

            for blk in range(NBLK):
                # G_f = (emb @ w[f]) for this block, computed during the
                # stream (fills PE bubbles), cast to bf16 in SBUF.
                gsb = gsbp.tile([128, K * OUT], bf16, tag="gsb")
                # 16 group stages, each [1, 512] = 8 atoms x 64 k
                stage = stagep.tile([1, 16 * 512], f32, tag="stage")

                for g in range(16):
                    t = blk * 16 + g
                    et = expp.tile([128, 8 * 1024], bf16, tag="exp")
                    nc.sync.dma_start(et[:], exp_d[t])
                    # 8 atoms share one PSUM bank; single start clears the
                    # whole 2 KiB zero region, per-address has_written turns
                    # each atom's first chunk into a write.
                    ps = ps1p.tile([128, 512], f32, tag="ps1")
                    for a in range(GA):
                        m = t * GA + a
                        for c in range(16):
                            nc.tensor.matmul(
                                ps[0:1, 64 * a : 64 * (a + 1)],
                                adjA[:, 16 * m + c : 16 * m + c + 1],
                                et[:, 1024 * a + 64 * c : 1024 * a + 64 * (c + 1)],
                                start=(a == 0 and c == 0),
                                stop=(a == GA - 1 and c == 15),
                            )
                    nc.scalar.copy(stage[0:1, 512 * g : 512 * (g + 1)], ps[0:1, :])
                    # interleave four G_f matmuls per group
                    for f in range(4 * g, 4 * g + 4):
                        g2 = ps2p.tile([128, OUT], f32, tag="ps2")
                        nc.tensor.matmul(
                            g2[:, :],
                            embT[:, 128 * blk : 128 * (blk + 1)],
                            wsb[:, OUT * f : OUT * (f + 1)],
                            start=True,
                            stop=True,
                        )
                        nc.vector.tensor_copy(gsb[:, OUT * f : OUT * (f + 1)], g2[:, :])

                # ---- step 2 for this block of 128 atoms ----
                # transpose bounce through DRAM on the gpsimd queue (the sync
                # queue must stay pure exp stream)
                nc.gpsimd.dma_start(scratch_d[blk : blk + 1, :], stage[0:1, :])
                aexp = aexpp.tile([128, K], f32, tag="aexp")
                nc.gpsimd.dma_start(
                    aexp[:],
                    scratch_d[blk : blk + 1, :].rearrange("b (p k) -> (b p) k", k=K),
                )
                # four interleaved DVE scale-accumulate chains over f
                accs = [None] * 4
                for r in range(16):
                    for ci in range(4):
                        f = 4 * r + ci
                        nacc = accp.tile([128, OUT], f32, tag=f"acc{ci}")
                        if r == 0:
                            nc.vector.tensor_scalar_mul(
                                nacc[:], gsb[:, OUT * f : OUT * (f + 1)],
                                aexp[:, f : f + 1],
                            )
                        else:
                            nc.vector.scalar_tensor_tensor(
                                nacc[:],
                                gsb[:, OUT * f : OUT * (f + 1)],
                                aexp[:, f : f + 1],
                                accs[ci][:],
                                mybir.AluOpType.mult,
                                mybir.AluOpType.add,
                            )
                        accs[ci] = nacc
                s01 = accp.tile([128, OUT], f32, tag="acc0")
                nc.vector.tensor_add(s01[:], accs[0][:], accs[1][:])
                s23 = accp.tile([128, OUT], f32, tag="acc1")
                nc.vector.tensor_add(s23[:], accs[2][:], accs[3][:])
                acc = accp.tile([128, OUT], f32, tag="acc2")
                nc.vector.tensor_add(acc[:], s01[:], s23[:])
                # softplus(x) = relu(x) + ln(1 + exp(-min(|x|, 87))); abs/exp/
                # ln on ScalarE (one table, pre-warmed), min/relu/adds on DVE.
                t_abs = outp.tile([128, OUT], f32, tag="outp")
                nc.scalar.activation(
                    t_abs[:], acc[:], mybir.ActivationFunctionType.Abs
                )
                t_cl = outp.tile([128, OUT], f32, tag="outp")
                nc.vector.tensor_scalar_min(t_cl[:], t_abs[:], 87.0)
                t_exp = outp.tile([128, OUT], f32, tag="outp")
                nc.scalar.activation(
                    t_exp[:], t_cl[:], mybir.ActivationFunctionType.Exp, scale=-1.0
                )
                t_ln = outp.tile([128, OUT], f32, tag="outp")
                nc.scalar.activation(
                    t_ln[:], t_exp[:], mybir.ActivationFunctionType.Ln, bias=1.0
                )
                t_relu = outp.tile([128, OUT], f32, tag="outp")
                nc.vector.tensor_scalar_max(t_relu[:], acc[:], 0.0)
                t_s = outp.tile([128, OUT], f32, tag="outp")
                nc.vector.tensor_add(t_s[:], t_ln[:], t_relu[:])
                ot = outp.tile([128, OUT], f32, tag="outp")
                nc.vector.tensor_add(ot[:], t_s[:], biassb[:])
                nc.scalar.dma_start(out_d[128 * blk : 128 * (blk + 1), :], ot[:])

    nc.compile()
    return nc


def _prep_inputs(dist_adj, dist_exp, atom_emb, bilinear_w, bilinear_b):
    dist_adj = np.asarray(dist_adj, dtype=np.float32)
    dist_exp = np.asarray(dist_exp, dtype=np.float32)
    atom_emb = np.asarray(atom_emb, dtype=np.float32)
    bilinear_w = np.asarray(bilinear_w, dtype=np.float32)
    bilinear_b = np.asarray(bilinear_b, dtype=np.float32)

    # [core, t, p, aq]: groups of 8 atoms; per partition 16 KiB contiguous.
    # aq = 1024a + 64c + k, n = 16p + c.
    exp_b = (
        dist_exp.astype(_BF)
        .reshape(N_CORES, NG, GA, 128, 1024)
        .transpose(0, 1, 3, 2, 4)
        .reshape(N_CORES, NG, 128, 8192)
    )
    # adjA[core, j, 16m + c] = dist_adj[core*M + m, 16j + c]
    adjA = (
        dist_adj.reshape(N_CORES, M, 128, 16)
        .transpose(0, 2, 1, 3)
        .reshape(N_CORES, 128, 16 * M)
        .astype(_BF, order="C")
    )
    embT = atom_emb.reshape(N_CORES, M, H).transpose(0, 2, 1).astype(_BF, order="C")
    w2 = bilinear_w.transpose(1, 0, 2).reshape(H, K * OUT).astype(_BF, order="C")
    biasb = np.ascontiguousarray(
        np.broadcast_to(bilinear_b.astype(np.float32), (128, OUT))
    )

    in_maps = []
    for i in range(N_CORES):
        in_maps.append(
            {
                "exp": np.ascontiguousarray(exp_b[i]),
                "adjA": np.ascontiguousarray(adjA[i]),
                "embT": np.ascontiguousarray(embT[i]),
                "w": w2,
                "bias": biasb,
            }
        )
    return in_maps


def _run(in_maps, **kwargs):
    _ensure_path()
    from concourse.bass_utils import run_bass_kernel_spmd

    if "nc" not in _CACHE:
        _CACHE["nc"] = _build()
    nc = _CACHE["nc"]
    res = run_bass_kernel_spmd(nc, in_maps, core_ids=list(range(N_CORES)), **kwargs)
    return res


def kernel(dist_adj, dist_exp, atom_emb, bilinear_w, bilinear_b):
    in_maps = _prep_inputs(dist_adj, dist_exp, atom_emb, bilinear_w, bilinear_b)
    res = _run(in_maps)
    out = np.concatenate(
        [np.asarray(res.results[i]["out"]) for i in range(N_CORES)], axis=0
    )
    return out.astype(np.float32)


# revision 11
# speedup vs baseline: 1.0079x; 1.0079x over previous
# Trainium2 Bass kernel for AtomTypeGNN message passing.
#
#   adj_exp[m,k] = sum_n dist_adj[m,n] * dist_exp[m,n,k]          (streams 1 GiB)
#   feat[m,o]    = sum_{f,h} adj_exp[m,f] * w[f,h,o] * emb[m,h]
#   out          = softplus(feat) + b
#
# Output row m depends only on row m of the inputs -> pure data parallel over
# atoms, 8 NeuronCores, 256 atoms each, no collectives.
#
# Device strategy (per core), v2:
#   Step 1: atoms stream in groups of 8; one 2 MiB DMA per group on the sync
#     (SP HWDGE) queue, which carries NOTHING else, so no cross-phase
#     dependency ever stalls the exp stream (v1 lost ~15us to the aexp load
#     parked between blocks in the SP FIFO).  All 8 atoms of a group
#     accumulate into ONE PSUM bank at different free offsets: start=True on
#     the group's first matmul marks the whole 2 KiB zero region pending-zero,
#     so each atom's first chunk writes and later chunks accumulate
#     (has_written semantics).  One ScalarE evac [1,512] per group replaces
#     v1's four [1,64] copies (ScalarE busy ~115us -> ~22us).
#   Step 2: per 128-atom block, ONE scratch store + ONE gather load (both on
#     the gpsimd SWDGE queue) transpose adj_exp to [128 atoms, 64].  The
#     f-contraction feat = sum_f aexp[:,f] * G_f runs as FOUR interleaved
#     DVE scalar_tensor_tensor chains (dep distance 4 hides DVE latency);
#     G_f = emb @ w[f] is computed on the PE during the stream.  Softplus
#     splits between ScalarE (abs/exp/ln in one act table, pre-warmed at
#     kernel start so no table load lands in the tail) and DVE (min/relu/
#     adds).  Output DMAs ride the scalar (ACT HWDGE) queue: gpsimd's
#     end-of-kernel DRAIN cost ~7us in v1.
#
# Inputs are pre-swizzled/cast on the host (bf16 compute, f32 accumulate:
# ~3e-3 relative error, memory roofline halves to ~190us/core).

import numpy as np
import ml_dtypes

N = 2048
K = 64
H = 128
OUT = 128
N_CORES = 8
M = N // N_CORES  # 256 atoms per core
GA = 8            # atoms per group / per PSUM bank
NG = M // GA      # 32 groups per core
NBLK = M // 128   # 2 step-2 blocks per core

_BF = ml_dtypes.bfloat16

_CACHE = {}


def _ensure_path():
    import sys

    for p in ("/opt/trn_rl_repo",):
        if p not in sys.path:
            sys.path.insert(0, p)


def _build():
    _ensure_path()
    import concourse.bass as bass  # noqa: F401
    import concourse.tile as tile
    from concourse import bacc, mybir

    f32 = mybir.dt.float32
    bf16 = mybir.dt.bfloat16
    fp16 = mybir.dt.float16

    nc = bacc.Bacc(
        "TRN2",
        target_bir_lowering=False,
        debug=False,
        num_devices=N_CORES,
    )

    # [t, p, aq]: atom group t = atoms 8t..8t+7, partition p, aq = 1024*a + q,
    # q = 64*c + k, n = 16p + c.  Per partition 16 KiB contiguous in DRAM.
    exp_d = nc.declare_dram_parameter("exp", [NG, 128, 8 * 1024], bf16, isOutput=False)
    # adjA[j, 16m + c] = dist_adj[m, 16j + c]
    adjA_d = nc.declare_dram_parameter("adjA", [128, 16 * M], bf16, isOutput=False)
    # embT[h, m]
    embT_d = nc.declare_dram_parameter("embT", [H, M], bf16, isOutput=False)
    # w2[h, 128f + o] = bilinear_w[f, h, o]
    w_d = nc.declare_dram_parameter("w", [H, K * OUT], bf16, isOutput=False)
    # bias broadcast to all partitions
    bias_d = nc.declare_dram_parameter("bias", [128, OUT], f32, isOutput=False)
    out_d = nc.declare_dram_parameter("out", [M, OUT], f32, isOutput=True)

    # adj_exp bounce buffer, one row per block: [g*512 + a*64 + k]
    scratch_d = nc.dram_tensor("scratch", [NBLK, 16 * 512], fp16)

    with tile.TileContext(nc) as tc:
        with (
            tc.tile_pool(name="const", bufs=1) as constp,
            tc.tile_pool(name="exp", bufs=6) as expp,
            tc.tile_pool(name="ps1", bufs=3, space="PSUM") as ps1p,
            tc.tile_pool(name="stage", bufs=1) as stagep,
            tc.tile_pool(name="aexp", bufs=2) as aexpp,
            tc.tile_pool(name="ps2", bufs=3, space="PSUM") as ps2p,
            tc.tile_pool(name="gsb", bufs=2) as gsbp,
            tc.tile_pool(name="acc", bufs=10) as accp,
            tc.tile_pool(name="outp", bufs=6) as outp,
        ):
            # consts at the HEAD of the sync queue: they must land at full
            # rate before the stream floods HBM (on the scalar queue they
            # trickled at ~70 GB/s against the saturated stream and the PE
            # sat idle 20us waiting for adjA).
            biassb = constp.tile([128, OUT], f32, tag="bias")
            nc.sync.dma_start(biassb[:], bias_d[:, :])
            adjA = constp.tile([128, 16 * M], bf16, tag="adjA")
            nc.sync.dma_start(adjA[:], adjA_d[:, :])
            wsb = constp.tile([128, K * OUT], bf16, tag="wsb")
            nc.sync.dma_start(wsb[:], w_d[:, :])
            embT = constp.tile([128, M], bf16, tag="embT")
            nc.sync.dma_start(embT[:], embT_d[:, :])

            # Warm the natural_log_exp act table (abs/exp/ln/relu/copy share
            # it) before the first evac copy, so no ACT_TABLE_LOAD lands in
            # the tail's critical path.
            warm = constp.tile([1, 2], f32, tag="warm")
            nc.scalar.activation(
                warm[0:1, :], biassb[0:1, 0:2], mybir.ActivationFunctionType.Abs
            )

            for blk in range(NBLK):
                # G_f = (emb @ w[f]) for this block, computed during the
                # stream (fills PE bubbles).  fp16 (not bf16) for the whole
                # step-2 path: 11 mantissa bits instead of 8 AND 2x DVE
                # throughput for the tail chains.
                gsb = gsbp.tile([128, K * OUT], fp16, tag="gsb")
                # 16 group stages, each [1, 512] = 8 atoms x 64 k
                stage = stagep.tile([1, 16 * 512], fp16, tag="stage")

                for g in range(16):
                    t = blk * 16 + g
                    et = expp.tile([128, 8 * 1024], bf16, tag="exp")
                    nc.sync.dma_start(et[:], exp_d[t])
                    # 8 atoms share one PSUM bank; single start clears the
                    # whole 2 KiB zero region, per-address has_written turns
                    # each atom's first chunk into a write.
                    ps = ps1p.tile([128, 512], f32, tag="ps1")
                    for a in range(GA):
                        m = t * GA + a
                        for c in range(16):
                            nc.tensor.matmul(
                                ps[0:1, 64 * a : 64 * (a + 1)],
                                adjA[:, 16 * m + c : 16 * m + c + 1],
                                et[:, 1024 * a + 64 * c : 1024 * a + 64 * (c + 1)],
                                start=(a == 0 and c == 0),
                                stop=(a == GA - 1 and c == 15),
                            )
                    nc.scalar.copy(stage[0:1, 512 * g : 512 * (g + 1)], ps[0:1, :])
                    # interleave four G_f matmuls per group
                    for f in range(4 * g, 4 * g + 4):
                        g2 = ps2p.tile([128, OUT], f32, tag="ps2")
                        nc.tensor.matmul(
                            g2[:, :],
                            embT[:, 128 * blk : 128 * (blk + 1)],
                            wsb[:, OUT * f : OUT * (f + 1)],
                            start=True,
                            stop=True,
                        )
                        nc.vector.tensor_copy(gsb[:, OUT * f : OUT * (f + 1)], g2[:, :])

                # ---- step 2 for this block of 128 atoms ----
                # transpose bounce through DRAM on the gpsimd queue (the sync
                # queue must stay pure exp stream)
                nc.gpsimd.dma_start(scratch_d[blk : blk + 1, :], stage[0:1, :])
                # f32: DVE tensor_scalar ops require a float32 scalar operand;
                # the SWDGE load casts fp16 -> f32 in flight.
                aexp = aexpp.tile([128, K], f32, tag="aexp")
                nc.gpsimd.dma_start(
                    aexp[:],
                    scratch_d[blk : blk + 1, :].rearrange("b (p k) -> (b p) k", k=K),
                )
                # four interleaved DVE scale-accumulate chains over f
                accs = [None] * 4
                for r in range(16):
                    for ci in range(4):
                        f = 4 * r + ci
                        nacc = accp.tile([128, OUT], fp16, tag=f"acc{ci}")
                        if r == 0:
                            nc.vector.tensor_scalar_mul(
                                nacc[:], gsb[:, OUT * f : OUT * (f + 1)],
                                aexp[:, f : f + 1],
                            )
                        else:
                            nc.vector.scalar_tensor_tensor(
                                nacc[:],
                                gsb[:, OUT * f : OUT * (f + 1)],
                                aexp[:, f : f + 1],
                                accs[ci][:],
                                mybir.AluOpType.mult,
                                mybir.AluOpType.add,
                            )
                        accs[ci] = nacc
                s01 = accp.tile([128, OUT], fp16, tag="acc0")
                nc.vector.tensor_add(s01[:], accs[0][:], accs[1][:])
                s23 = accp.tile([128, OUT], fp16, tag="acc1")
                nc.vector.tensor_add(s23[:], accs[2][:], accs[3][:])
                acc = accp.tile([128, OUT], f32, tag="acc2")
                nc.vector.tensor_add(acc[:], s01[:], s23[:])
                # softplus(x) = relu(x) + ln(1 + exp(-min(|x|, 87))); abs/exp/
                # ln on ScalarE (one table, pre-warmed), min/relu/adds on DVE.
                t_abs = outp.tile([128, OUT], f32, tag="outp")
                nc.scalar.activation(
                    t_abs[:], acc[:], mybir.ActivationFunctionType.Abs
                )
                t_cl = outp.tile([128, OUT], f32, tag="outp")
                nc.vector.tensor_scalar_min(t_cl[:], t_abs[:], 87.0)
                t_exp = outp.tile([128, OUT], f32, tag="outp")
                nc.scalar.activation(
                    t_exp[:], t_cl[:], mybir.ActivationFunctionType.Exp, scale=-1.0
                )
                t_ln = outp.tile([128, OUT], f32, tag="outp")
                nc.scalar.activation(
                    t_ln[:], t_exp[:], mybir.ActivationFunctionType.Ln, bias=1.0
                )
                t_relu = outp.tile([128, OUT], f32, tag="outp")
                nc.vector.tensor_scalar_max(t_relu[:], acc[:], 0.0)
                t_s = outp.tile([128, OUT], f32, tag="outp")
                nc.vector.tensor_add(t_s[:], t_ln[:], t_relu[:])
                ot = outp.tile([128, OUT], f32, tag="outp")
                nc.vector.tensor_add(ot[:], t_s[:], biassb[:])
                nc.scalar.dma_start(out_d[128 * blk : 128 * (blk + 1), :], ot[:])

    nc.compile()
    return nc


def _prep_inputs(dist_adj, dist_exp, atom_emb, bilinear_w, bilinear_b):
    dist_adj = np.asarray(dist_adj, dtype=np.float32)
    dist_exp = np.asarray(dist_exp, dtype=np.float32)
    atom_emb = np.asarray(atom_emb, dtype=np.float32)
    bilinear_w = np.asarray(bilinear_w, dtype=np.float32)
    bilinear_b = np.asarray(bilinear_b, dtype=np.float32)

    # [core, t, p, aq]: groups of 8 atoms; per partition 16 KiB contiguous.
    # aq = 1024a + 64c + k, n = 16p + c.
    exp_b = (
        dist_exp.astype(_BF)
        .reshape(N_CORES, NG, GA, 128, 1024)
        .transpose(0, 1, 3, 2, 4)
        .reshape(N_CORES, NG, 128, 8192)
    )
    # adjA[core, j, 16m + c] = dist_adj[core*M + m, 16j + c]
    adjA = (
        dist_adj.reshape(N_CORES, M, 128, 16)
        .transpose(0, 2, 1, 3)
        .reshape(N_CORES, 128, 16 * M)
        .astype(_BF, order="C")
    )
    embT = atom_emb.reshape(N_CORES, M, H).transpose(0, 2, 1).astype(_BF, order="C")
    w2 = bilinear_w.transpose(1, 0, 2).reshape(H, K * OUT).astype(_BF, order="C")
    biasb = np.ascontiguousarray(
        np.broadcast_to(bilinear_b.astype(np.float32), (128, OUT))
    )

    in_maps = []
    for i in range(N_CORES):
        in_maps.append(
            {
                "exp": np.ascontiguousarray(exp_b[i]),
                "adjA": np.ascontiguousarray(adjA[i]),
                "embT": np.ascontiguousarray(embT[i]),
                "w": w2,
                "bias": biasb,
            }
        )
    return in_maps


def _run(in_maps, **kwargs):
    _ensure_path()
    from concourse.bass_utils import run_bass_kernel_spmd

    if "nc" not in _CACHE:
        _CACHE["nc"] = _build()
    nc = _CACHE["nc"]
    res = run_bass_kernel_spmd(nc, in_maps, core_ids=list(range(N_CORES)), **kwargs)
    return res


def kernel(dist_adj, dist_exp, atom_emb, bilinear_w, bilinear_b):
    in_maps = _prep_inputs(dist_adj, dist_exp, atom_emb, bilinear_w, bilinear_b)
    res = _run(in_maps)
    out = np.concatenate(
        [np.asarray(res.results[i]["out"]) for i in range(N_CORES)], axis=0
    )
    return out.astype(np.float32)
